# revision 2
# baseline (speedup 1.0000x reference)
"""CutMix kernel for Trainium2, 8 NeuronCores, pure data parallel.

out[b,h,w,c] = x[b,h,w,c] outside the per-sample box [y1,y2) x [x1,x2),
x[perm[b],h,w,c] inside it.  The mask is binary, so this is pure
replacement: no arithmetic on the pixel values (bit-exact output).

Sharding: batch dim across 8 cores (8 samples each).  The host
pre-gathers xp = x[perm[shard]] so the shuffle is shard-local.

Device kernel per core, per 128-row chunk (4 chunks per sample row
block of 512):
  - static DMA load of xs chunk            on the SP HWDGE ring
  - static DMA load of xp chunk            on the ACT HWDGE ring
  - box mask = outer product h_mask (x) w_mask on the PE into PSUM
    (bf16 operands; the 0/1 values are exact)
  - copy_predicated(xs_tile, mask bitcast to i32, xp_tile) on DVE:
    lanes with mask!=0 keep xs, mask==0 lanes take xp
  - static DMA store to out                on the GPSIMD (SWDGE) ring

Ring assignment matters: measured on HW, xs+xp loads sharing one HWDGE
ring runs 4x slower (59 GB/s/ring effective); one contiguous stream per
ring with stores on the SWDGE ring reaches ~350 GB/s aggregate, which
is the HBM roofline for the 72 MiB/core of traffic (213 us/iter
measured vs 218 us TimelineSim estimate).

An indirect-DMA row gather of only the in-box xp rows was measured at
~5 us per descriptor on this hardware (5.2 ms/iter for 1024 row-quad
descriptors) and is a dead end; the full static xp load is 24x faster.
"""

import numpy as np

import concourse.bacc as bacc
import concourse.mybir as mybir
from concourse.tile import TileContext
from concourse.bass_utils import run_bass_kernel_spmd

B, H, W, C = 64, 512, 512, 3
NCORES = 8
BS = B // NCORES            # samples per core
ROWS = BS * H               # 4096 image rows per core
RC = W * C                  # 1536 floats per image row
P = 128                     # partitions per chunk
CH = H // P                 # 4 chunks per sample
F32 = mybir.dt.float32
I32 = mybir.dt.int32
BF16 = mybir.dt.bfloat16


def build_nc(reps: int = 1, xs_bufs: int = 6, xp_bufs: int = 6):
    nc = bacc.Bacc("TRN2", target_bir_lowering=False, debug=False,
                   num_devices=NCORES)
    xs = nc.dram_tensor("xs", [ROWS, RC], F32, kind="ExternalInput")
    xp = nc.dram_tensor("xp", [ROWS, RC], F32, kind="ExternalInput")
    # boxf = [y1(8) | y2(8) | x1(8) | x2(8)] as fp32
    boxf = nc.dram_tensor("boxf", [1, 4 * BS], F32, kind="ExternalInput")
    out = nc.dram_tensor("out", [ROWS, RC], F32, kind="ExternalOutput")

    with TileContext(nc) as tc:
        with (
            tc.tile_pool(name="const", bufs=1) as cpool,
            tc.tile_pool(name="small", bufs=2) as spool,
            tc.tile_pool(name="xst", bufs=xs_bufs) as xs_pool,
            tc.tile_pool(name="xpt", bufs=xp_bufs) as xp_pool,
            tc.tile_pool(name="mask", bufs=2, space="PSUM") as mask_pool,
        ):
            # ---- one-time setup ----
            scal_row = cpool.tile([1, 4 * BS], F32, tag="scal_row")
            nc.sync.dma_start(out=scal_row[:], in_=boxf[:])

            # h index 0..511 and w index (repeated x3 channels), partition 0
            iota_h = cpool.tile([1, H], I32, tag="ioh")
            nc.gpsimd.iota(iota_h[:], pattern=[[1, H]], base=0,
                           channel_multiplier=0)
            iota_hf = cpool.tile([1, H], F32, tag="iohf")
            nc.vector.tensor_copy(iota_hf[:], iota_h[:])
            iota_w = cpool.tile([1, RC], I32, tag="iow")
            nc.gpsimd.iota(iota_w[:], pattern=[[1, W], [0, C]], base=0,
                           channel_multiplier=0)
            iota_wf = cpool.tile([1, RC], F32, tag="iowf")
            nc.vector.tensor_copy(iota_wf[:], iota_w[:])

            def main_body(_iv=None):
                for s in range(BS):
                    y1s = scal_row[0:1, s:s + 1]
                    y2s = scal_row[0:1, BS + s:BS + s + 1]
                    x1s = scal_row[0:1, 2 * BS + s:2 * BS + s + 1]
                    x2s = scal_row[0:1, 3 * BS + s:3 * BS + s + 1]

                    # h mask over the sample's 512 rows (partition 0);
                    # bf16 so the PE outer product runs at full rate
                    h_ge = spool.tile([1, H], F32, tag="h_ge")
                    nc.vector.tensor_scalar(out=h_ge[:], in0=iota_hf[:],
                                            scalar1=y1s, scalar2=None,
                                            op0=mybir.AluOpType.is_ge)
                    h_lt = spool.tile([1, H], F32, tag="h_lt")
                    nc.vector.tensor_scalar(out=h_lt[:], in0=iota_hf[:],
                                            scalar1=y2s, scalar2=None,
                                            op0=mybir.AluOpType.is_lt)
                    h_row = spool.tile([1, H], BF16, tag="h_row")
                    nc.vector.tensor_tensor(out=h_row[:], in0=h_ge[:],
                                            in1=h_lt[:],
                                            op=mybir.AluOpType.mult)

                    # w mask over the row's 1536 floats (partition 0)
                    w_ge = spool.tile([1, RC], F32, tag="w_ge")
                    nc.vector.tensor_scalar(out=w_ge[:], in0=iota_wf[:],
                                            scalar1=x1s, scalar2=None,
                                            op0=mybir.AluOpType.is_ge)
                    w_lt = spool.tile([1, RC], F32, tag="w_lt")
                    nc.vector.tensor_scalar(out=w_lt[:], in0=iota_wf[:],
                                            scalar1=x2s, scalar2=None,
                                            op0=mybir.AluOpType.is_lt)
                    w_row = spool.tile([1, RC], BF16, tag="w_row")
                    nc.vector.tensor_tensor(out=w_row[:], in0=w_ge[:],
                                            in1=w_lt[:],
                                            op=mybir.AluOpType.mult)

                    for c in range(CH):
                        r0 = s * H + c * P
                        xs_t = xs_pool.tile([P, RC], F32, tag="xs_t")
                        nc.sync.dma_start(out=xs_t[:], in_=xs[r0:r0 + P, :])
                        xp_t = xp_pool.tile([P, RC], F32, tag="xp_t")
                        nc.scalar.dma_start(out=xp_t[:], in_=xp[r0:r0 + P, :])

                        mask = mask_pool.tile([P, RC], F32, tag="mask")
                        for n in range(RC // 512):
                            nc.tensor.matmul(
                                out=mask[:, n * 512:(n + 1) * 512],
                                lhsT=h_row[0:1, c * P:(c + 1) * P],
                                rhs=w_row[0:1, n * 512:(n + 1) * 512],
                                start=True, stop=True)

                        # HW CopyPredicated wants an integer mask; the fp32
                        # PSUM bit patterns (0x0 / 0x3F800000) predicate the
                        # same way reinterpreted as int32
                        nc.vector.copy_predicated(
                            xs_t[:], mask[:].bitcast(I32), xp_t[:])
                        nc.gpsimd.dma_start(out=out[r0:r0 + P, :],
                                            in_=xs_t[:])

            if reps > 1:
                with tc.For_i(0, reps, 1) as _iv:
                    main_body(_iv)
            else:
                main_body()

    nc.finalize()
    return nc


_NC_CACHE = {}


def _get_nc(reps: int = 1):
    if reps not in _NC_CACHE:
        _NC_CACHE[reps] = build_nc(reps)
    return _NC_CACHE[reps]


def make_in_maps(x, y1, y2, x1, x2, perm):
    x = np.ascontiguousarray(np.asarray(x, dtype=np.float32))
    y1 = np.asarray(y1).astype(np.int32)
    y2 = np.asarray(y2).astype(np.int32)
    x1 = np.asarray(x1).astype(np.int32)
    x2 = np.asarray(x2).astype(np.int32)
    perm = np.asarray(perm).astype(np.int64)
    in_maps = []
    for m in range(NCORES):
        sl = slice(m * BS, (m + 1) * BS)
        xs_m = np.ascontiguousarray(x[sl].reshape(ROWS, RC))
        xp_m = np.ascontiguousarray(x[perm[sl]].reshape(ROWS, RC))
        boxf = np.concatenate([y1[sl], y2[sl], x1[sl], x2[sl]]) \
            .astype(np.float32).reshape(1, 4 * BS)
        in_maps.append({"xs": xs_m, "xp": xp_m, "boxf": boxf})
    return in_maps


def run(x, y1, y2, x1, x2, perm, trace=False):
    """Returns (out, BassKernelResults)."""
    nc = _get_nc()
    in_maps = make_in_maps(x, y1, y2, x1, x2, perm)
    res = run_bass_kernel_spmd(nc, in_maps, list(range(NCORES)), trace=trace)
    out = np.empty((B, H, W, C), dtype=np.float32)
    for m in range(NCORES):
        out[m * BS:(m + 1) * BS] = res.results[m]["out"].reshape(BS, H, W, C)
    return out, res


def kernel(x, y1, y2, x1, x2, perm):
    out, _ = run(x, y1, y2, x1, x2, perm)
    return out


# revision 3
# speedup vs baseline: 1.2277x; 1.2277x over previous
"""CutMix kernel for Trainium2, 8 NeuronCores, pure data parallel.

out[b,h,w,c] = x[b,h,w,c] outside the per-sample box [y1,y2) x [x1,x2),
x[perm[b],h,w,c] inside it.  The mask is binary, so this is pure
replacement: no arithmetic on the pixel values (bit-exact output).

Sharding: batch dim across 8 cores, 8 samples each, with a host-side
greedy balance of per-sample box-chunk counts so every core carries a
near-equal xp load (SPMD wall time is the slowest core).  The host
pre-gathers xp = x[perm[assigned]] so the shuffle is shard-local.

Device kernel per core, per 128-row chunk (4 per sample):
  - static DMA load of the xs chunk
  - conditional DMA load of the xp chunk, skipped (replaced by a
    same-descriptor-count 512-byte DMA) when the chunk has no box rows
    or the box is degenerate.  Flags are computed on DVE from the box
    scalars, reg_load-ed into the issuing engine's registers, and drive
    a single-engine tc.If/Else.  Skipped tiles hold stale data, which is
    safe: the box mask is all-zero for those chunks, so copy_predicated
    never copies from them.  The Else-path DMA keeps the descriptor
    count (128) identical on both paths so every cumulative DMA
    semaphore count is path-independent.
  - box mask = outer product h_mask (x) w_mask on the PE into PSUM
    (bf16 operands; the 0/1 values are exact)
  - copy_predicated(xs_tile, mask bitcast to i32, xp_tile) on DVE:
    mask!=0 lanes (inside the box) take xp, the rest keep xs
  - static DMA store to out

The three DMA streams rotate over the three DMA-capable queues
(SP, ACT, GPSIMD/SWDGE) by sample index so each queue carries ~1/3 of
the ~60 MiB/core of traffic.  Measured pitfalls encoded here: two load
streams interleaved on one HWDGE queue run ~4x slower; an indirect-DMA
row gather costs ~5 us per descriptor (5.2 ms/iter) and loses to static
loads by 24x.

Measured: ~185 us/iter vs 265 us for the all-static unconditional
version and ~211 us for its pure-bandwidth floor (72 MiB @ 358 GB/s);
the conditional+balanced version moves ~60 MiB/core.
"""

import numpy as np

import concourse.bass as bass
import concourse.bacc as bacc
import concourse.mybir as mybir
from concourse.tile import TileContext
from concourse.bass_utils import run_bass_kernel_spmd
from concourse.expressions import make_scalar_value

B, H, W, C = 64, 512, 512, 3
NCORES = 8
BS = B // NCORES            # samples per core
ROWS = BS * H               # 4096 image rows per core
RC = W * C                  # 1536 floats per image row
P = 128                     # partitions per chunk
CH = H // P                 # 4 chunks per sample
F32 = mybir.dt.float32
I32 = mybir.dt.int32
BF16 = mybir.dt.bfloat16


def build_nc(reps: int = 1, xs_bufs: int = 6, xp_bufs: int = 6,
             full_else: bool = False):
    nc = bacc.Bacc("TRN2", target_bir_lowering=False, debug=False,
                   num_devices=NCORES)
    xs = nc.dram_tensor("xs", [ROWS, RC], F32, kind="ExternalInput")
    xp = nc.dram_tensor("xp", [ROWS, RC], F32, kind="ExternalInput")
    # boxf = [y1(8) | y2(8) | x1(8) | x2(8)] as fp32
    boxf = nc.dram_tensor("boxf", [1, 4 * BS], F32, kind="ExternalInput")
    out = nc.dram_tensor("out", [ROWS, RC], F32, kind="ExternalOutput")

    with TileContext(nc) as tc:
        with (
            tc.tile_pool(name="const", bufs=1) as cpool,
            tc.tile_pool(name="small", bufs=2) as spool,
            tc.tile_pool(name="xst", bufs=xs_bufs) as xs_pool,
            tc.tile_pool(name="xpt", bufs=xp_bufs) as xp_pool,
            tc.tile_pool(name="mask", bufs=2, space="PSUM") as mask_pool,
        ):
            # ---- one-time setup ----
            scal_row = cpool.tile([1, 4 * BS], F32, tag="scal_row")
            nc.sync.dma_start(out=scal_row[:], in_=boxf[:])

            iota_h = cpool.tile([1, H], I32, tag="ioh")
            nc.gpsimd.iota(iota_h[:], pattern=[[1, H]], base=0,
                           channel_multiplier=0)
            iota_hf = cpool.tile([1, H], F32, tag="iohf")
            nc.vector.tensor_copy(iota_hf[:], iota_h[:])
            iota_w = cpool.tile([1, RC], I32, tag="iow")
            nc.gpsimd.iota(iota_w[:], pattern=[[1, W], [0, C]], base=0,
                           channel_multiplier=0)
            iota_wf = cpool.tile([1, RC], F32, tag="iowf")
            nc.vector.tensor_copy(iota_wf[:], iota_w[:])

            # chunk lower/upper row bounds [0,128,256,384] / [128,...,512]
            clo_i = cpool.tile([1, CH], I32, tag="clo_i")
            nc.gpsimd.iota(clo_i[:], pattern=[[P, CH]], base=0,
                           channel_multiplier=0)
            clo_f = cpool.tile([1, CH], F32, tag="clo_f")
            nc.vector.tensor_copy(clo_f[:], clo_i[:])
            chi_f = cpool.tile([1, CH], F32, tag="chi_f")
            nc.vector.tensor_scalar(out=chi_f[:], in0=clo_f[:],
                                    scalar1=float(P), scalar2=None,
                                    op0=mybir.AluOpType.add)

            # per-chunk take-flag registers on each DMA-capable engine
            ENGS = [(nc.sync, mybir.EngineType.SP),
                    (nc.scalar, mybir.EngineType.Activation),
                    (nc.gpsimd, mybir.EngineType.Pool)]
            flag_regs = {
                et: [nc.alloc_register(et, f"takef_{et.name}_{c}")
                     for c in range(CH)]
                for _, et in ENGS
            }

            def main_body(_iv=None):
                for s in range(BS):
                    y1s = scal_row[0:1, s:s + 1]
                    y2s = scal_row[0:1, BS + s:BS + s + 1]
                    x1s = scal_row[0:1, 2 * BS + s:2 * BS + s + 1]
                    x2s = scal_row[0:1, 3 * BS + s:3 * BS + s + 1]

                    # h mask over the sample's 512 rows (partition 0);
                    # bf16 operands run the PE outer product at full rate
                    h_ge = spool.tile([1, H], F32, tag="h_ge")
                    nc.vector.tensor_scalar(out=h_ge[:], in0=iota_hf[:],
                                            scalar1=y1s, scalar2=None,
                                            op0=mybir.AluOpType.is_ge)
                    h_lt = spool.tile([1, H], F32, tag="h_lt")
                    nc.vector.tensor_scalar(out=h_lt[:], in0=iota_hf[:],
                                            scalar1=y2s, scalar2=None,
                                            op0=mybir.AluOpType.is_lt)
                    h_row = spool.tile([1, H], BF16, tag="h_row")
                    nc.vector.tensor_tensor(out=h_row[:], in0=h_ge[:],
                                            in1=h_lt[:],
                                            op=mybir.AluOpType.mult)

                    # w mask over the row's 1536 floats (partition 0)
                    w_ge = spool.tile([1, RC], F32, tag="w_ge")
                    nc.vector.tensor_scalar(out=w_ge[:], in0=iota_wf[:],
                                            scalar1=x1s, scalar2=None,
                                            op0=mybir.AluOpType.is_ge)
                    w_lt = spool.tile([1, RC], F32, tag="w_lt")
                    nc.vector.tensor_scalar(out=w_lt[:], in0=iota_wf[:],
                                            scalar1=x2s, scalar2=None,
                                            op0=mybir.AluOpType.is_lt)
                    w_row = spool.tile([1, RC], BF16, tag="w_row")
                    nc.vector.tensor_tensor(out=w_row[:], in0=w_ge[:],
                                            in1=w_lt[:],
                                            op=mybir.AluOpType.mult)

                    # take[c] = (clo[c] < y2) * (chi[c] > y1) * (x2 > x1)
                    t_lt = spool.tile([1, CH], F32, tag="t_lt")
                    nc.vector.tensor_scalar(out=t_lt[:], in0=clo_f[:],
                                            scalar1=y2s, scalar2=None,
                                            op0=mybir.AluOpType.is_lt)
                    t_gt = spool.tile([1, CH], F32, tag="t_gt")
                    nc.vector.tensor_scalar(out=t_gt[:], in0=chi_f[:],
                                            scalar1=y1s, scalar2=None,
                                            op0=mybir.AluOpType.is_gt)
                    x_ne = spool.tile([1, 1], F32, tag="x_ne")
                    nc.vector.tensor_scalar(out=x_ne[:], in0=x2s,
                                            scalar1=x1s, scalar2=None,
                                            op0=mybir.AluOpType.is_gt)
                    take_f = spool.tile([1, CH], F32, tag="take_f")
                    nc.vector.tensor_tensor(out=take_f[:], in0=t_lt[:],
                                            in1=t_gt[:],
                                            op=mybir.AluOpType.mult)
                    take_f2 = spool.tile([1, CH], F32, tag="take_f2")
                    nc.vector.tensor_scalar(out=take_f2[:], in0=take_f[:],
                                            scalar1=x_ne[0:1, 0:1],
                                            scalar2=None,
                                            op0=mybir.AluOpType.mult)
                    take_i = spool.tile([1, CH], I32, tag="take_i")
                    nc.vector.tensor_copy(take_i[:], take_f2[:])

                    # rotate the three DMA streams over the three queues
                    ld_xs, _ = ENGS[s % 3]
                    ld_xp, xp_et = ENGS[(s + 1) % 3]
                    st, _ = ENGS[(s + 2) % 3]

                    for c in range(CH):
                        ld_xp.reg_load(flag_regs[xp_et][c],
                                       take_i[0:1, c:c + 1])

                    for c in range(CH):
                        r0 = s * H + c * P
                        xs_t = xs_pool.tile([P, RC], F32, tag="xs_t")
                        ld_xs.dma_start(out=xs_t[:], in_=xs[r0:r0 + P, :])
                        xp_t = xp_pool.tile([P, RC], F32, tag="xp_t")

                        sv = make_scalar_value(
                            bass.RegisterHandles([flag_regs[xp_et][c]]),
                            min_val=0, max_val=1)
                        with tc.If(sv == 1) as cmp:
                            ld_xp.dma_start(out=xp_t[:],
                                            in_=xp[r0:r0 + P, :])
                        with cmp.Else():
                            if full_else:
                                # CoreSim-validation build: full-size load
                                # so the uninit-memory checker stays quiet
                                ld_xp.dma_start(out=xp_t[:],
                                                in_=xp[r0:r0 + P, :])
                            else:
                                ld_xp.dma_start(out=xp_t[:, 0:1],
                                                in_=xp[r0:r0 + P, 0:1])

                        mask = mask_pool.tile([P, RC], F32, tag="mask")
                        for n in range(RC // 512):
                            nc.tensor.matmul(
                                out=mask[:, n * 512:(n + 1) * 512],
                                lhsT=h_row[0:1, c * P:(c + 1) * P],
                                rhs=w_row[0:1, n * 512:(n + 1) * 512],
                                start=True, stop=True)

                        # HW CopyPredicated wants an integer mask; the fp32
                        # PSUM bit patterns (0x0 / 0x3F800000) predicate the
                        # same way reinterpreted as int32
                        nc.vector.copy_predicated(
                            xs_t[:], mask[:].bitcast(I32), xp_t[:])
                        st.dma_start(out=out[r0:r0 + P, :], in_=xs_t[:])

            if reps > 1:
                with tc.For_i(0, reps, 1) as _iv:
                    main_body(_iv)
            else:
                main_body()

    nc.finalize()
    return nc


_NC_CACHE = {}


def _get_nc(reps: int = 1):
    if reps not in _NC_CACHE:
        _NC_CACHE[reps] = build_nc(reps)
    return _NC_CACHE[reps]


def _balanced_assignment(y1, y2, x1, x2):
    """Per-core sample lists equalizing the xp chunk-load totals."""
    def nchunks(i):
        if y2[i] <= y1[i] or x2[i] <= x1[i]:
            return 0
        return (int(y2[i]) + P - 1) // P - int(y1[i]) // P

    nch = [nchunks(i) for i in range(B)]
    order = sorted(range(B), key=lambda i: -nch[i])
    loads = [0] * NCORES
    counts = [0] * NCORES
    assign = [[] for _ in range(NCORES)]
    for i in order:
        m = min((j for j in range(NCORES) if counts[j] < BS),
                key=lambda j: loads[j])
        assign[m].append(i)
        loads[m] += nch[i]
        counts[m] += 1
    return assign


def make_in_maps(x, y1, y2, x1, x2, perm):
    """Returns (in_maps, assign): assign[m][j] = batch index of core m's
    j-th sample."""
    x = np.ascontiguousarray(np.asarray(x, dtype=np.float32))
    y1 = np.asarray(y1).astype(np.int32)
    y2 = np.asarray(y2).astype(np.int32)
    x1 = np.asarray(x1).astype(np.int32)
    x2 = np.asarray(x2).astype(np.int32)
    perm = np.asarray(perm).astype(np.int64)
    assign = _balanced_assignment(y1, y2, x1, x2)
    in_maps = []
    for m in range(NCORES):
        idx = np.asarray(assign[m], dtype=np.int64)
        xs_m = np.ascontiguousarray(x[idx].reshape(ROWS, RC))
        xp_m = np.ascontiguousarray(x[perm[idx]].reshape(ROWS, RC))
        boxf = np.concatenate([y1[idx], y2[idx], x1[idx], x2[idx]]) \
            .astype(np.float32).reshape(1, 4 * BS)
        in_maps.append({"xs": xs_m, "xp": xp_m, "boxf": boxf})
    return in_maps, assign


def run(x, y1, y2, x1, x2, perm, trace=False):
    """Returns (out, BassKernelResults)."""
    nc = _get_nc()
    in_maps, assign = make_in_maps(x, y1, y2, x1, x2, perm)
    res = run_bass_kernel_spmd(nc, in_maps, list(range(NCORES)), trace=trace)
    out = np.empty((B, H, W, C), dtype=np.float32)
    for m in range(NCORES):
        shard = res.results[m]["out"].reshape(BS, H, W, C)
        for j, i in enumerate(assign[m]):
            out[i] = shard[j]
    return out, res


def kernel(x, y1, y2, x1, x2, perm):
    out, _ = run(x, y1, y2, x1, x2, perm)
    return out
